# revision 17
# baseline (speedup 1.0000x reference)
"""Causal attention (dense transformer block) on 8 Trainium2 NeuronCores.

Problem: x (4, 256, 64, 64) fp32; 1x1-conv q/kv projections; 8-head causal
attention over S = 64*64 = 4096 flattened pixels (head_dim 32); output
projection.  Full inputs in, full output out.

Sharding: 8 cores = 4 batches x 2 head-groups (4 heads each).  Each core
computes q/k/v projections for its head group, flash-style causal attention
(scores kept transposed: k-positions on partitions, q-positions on free dim,
so softmax denominators come out of the AV matmul via an appended ones
column), and a partial output projection.  Host sums the two head-group
partials per batch and adds the output bias.

Engine budget (per core, timeline-sim): ACT runs the softmax exp (~35.9M
elements) and q/k projection-bias evacuations; PE runs ~600k moving rows of
matmul; DVE runs causal-mask multiplies (bf16 2x mode), v-bias adds, and the
per-chunk normalization; Pool broadcasts softmax reciprocals.  q/k/es/v are
bf16: stationary weight loads run 4x faster than fp32r and DVE tensor ops hit
the 2-byte 2x mode; matmul accumulation stays fp32 in PSUM.
"""

import math
from contextlib import ExitStack

import numpy as np

import concourse.bass as bass
import concourse.tile as tile
from concourse import bacc, mybir

N_CORES = 8
N, C, HH, WW = 4, 256, 64, 64
S = HH * WW            # 4096
E = 256                # q/k width
O = 256                # v/out width
H = 8                  # heads
HD = E // H            # 32 head dim
HG = 4                 # heads per core
P = 128                # partitions
QC = 512               # q-chunk (matmul moving free dim)
KT = 128               # k-tile (contraction block for AV)
NQ = S // QC           # 8 q-chunks
ACT_W = 3              # k-tiles exp'd per ScalarE call (3 psum banks)
VW = 34                # v columns per head (32 dims + ones col + pad)

F32 = mybir.dt.float32
F32R = mybir.dt.float32r
BF16 = mybir.dt.bfloat16
I16 = mybir.dt.int16

# Softmax exp is the Activation-engine bottleneck (~35.9M elements/core), so
# exp windows are routed round-robin across engines per this pattern.  A =
# ScalarE exact exp; D = DVE bf16-Schraudolph fast exp (one
# scalar_tensor_tensor producing bf16 bits as int16; max rel err ~4%).
# GPSIMD cannot read PSUM, so Pool instead runs the SBUF-only mask multiplies
# and the normalization muls.
EXP_PATTERN = "AADAD"
W_BANKS = 2            # psum window width in 512-col banks
QK_BUFS = 3            # qk psum window buffers (W_BANKS*QK_BUFS + 2 <= 8)
# bits = ps * SCHRAUD_A + SCHRAUD_B, bitcast int16 -> bf16 == exp(ps*scale)
SCHRAUD_A = float(128.0 * math.log2(math.e) / math.sqrt(HD))
SCHRAUD_B = float(127 * 128 - 7.25)


def build_kernel(reps=1):
    nc = bacc.Bacc("TRN2", target_bir_lowering=False, debug=False,
                   num_devices=N_CORES)

    # Per-core inputs (same shapes on every core, different data).
    xf = nc.dram_tensor("xf", (C, S), F32, kind="ExternalInput").ap()
    wqT = nc.dram_tensor("wqT", (C, P), F32, kind="ExternalInput").ap()
    wkT = nc.dram_tensor("wkT", (C, P), F32, kind="ExternalInput").ap()
    wvT = nc.dram_tensor("wvT", (C, O), F32, kind="ExternalInput").ap()
    wpT = nc.dram_tensor("wpT", (2, P, P), F32, kind="ExternalInput").ap()
    bq = nc.dram_tensor("bq", (P, 1), F32, kind="ExternalInput").ap()
    bk = nc.dram_tensor("bk", (P, 1), F32, kind="ExternalInput").ap()
    bv = nc.dram_tensor("bv", (1, P), F32, kind="ExternalInput").ap()
    masks = nc.dram_tensor("masks", (4, P, QC), F32, kind="ExternalInput").ap()
    out = nc.dram_tensor("out", (O, S), F32, kind="ExternalOutput").ap()

    with tile.TileContext(nc) as tc:
        with ExitStack() as ctx:
            _emit(ctx, tc, nc, xf, wqT, wkT, wvT, wpT, bq, bk, bv, masks, out,
                  reps=reps)

    nc.compile()
    return nc


def _ap3(t2d, d1, n1, d2, n2):
    """[P, n1, n2] strided view of a 2-D SBUF AP (free strides d1, d2)."""
    return bass.AP(tensor=t2d.tensor, offset=t2d.offset,
                   ap=[t2d.ap[0], [d1, n1], [d2, n2]])


def _emit(ctx, tc, nc, xf, wqT, wkT, wvT, wpT, bq, bk, bv, masks, out,
          reps=1):
    scale = 1.0 / math.sqrt(HD)
    Exp = mybir.ActivationFunctionType.Exp
    Ident = mybir.ActivationFunctionType.Identity

    consts = ctx.enter_context(tc.tile_pool(name="consts", bufs=1))
    qk_ps = ctx.enter_context(tc.tile_pool(name="qk_ps", bufs=QK_BUFS, space="PSUM"))
    av_ps = ctx.enter_context(tc.tile_pool(name="av_ps", bufs=2, space="PSUM"))
    work = ctx.enter_context(tc.tile_pool(name="work", bufs=6))
    norm = ctx.enter_context(tc.tile_pool(name="norm", bufs=2))
    tmp = ctx.enter_context(tc.tile_pool(name="tmp", bufs=1))

    # ---- load constants / weights -------------------------------------
    # DMA-loaded data cannot feed FP32r matmuls directly; a compute-engine
    # copy with float32r output performs the required rounding.
    wq_ld = tmp.tile([P, 2, P], F32, tag="w")
    nc.sync.dma_start(out=wq_ld, in_=wqT.rearrange("(c p) m -> p c m", p=P))
    wq_sb = consts.tile([P, 2, P], F32R)
    nc.vector.tensor_copy(wq_sb, wq_ld)
    wk_ld = tmp.tile([P, 2, P], F32, tag="w")
    nc.sync.dma_start(out=wk_ld, in_=wkT.rearrange("(c p) m -> p c m", p=P))
    wk_sb = consts.tile([P, 2, P], F32R)
    nc.vector.tensor_copy(wk_sb, wk_ld)
    wv_ld = tmp.tile([P, 2, O], F32, tag="w")
    nc.sync.dma_start(out=wv_ld, in_=wvT.rearrange("(c p) m -> p c m", p=P))
    wv_sb = consts.tile([P, 2, O], F32R)
    nc.vector.tensor_copy(wv_sb, wv_ld)
    wp_ld = tmp.tile([P, 2, P], F32, tag="w")
    nc.sync.dma_start(out=wp_ld, in_=wpT.rearrange("m p n -> p m n"))
    wp_sb = consts.tile([P, 2, P], F32R)
    nc.vector.tensor_copy(wp_sb, wp_ld)
    # sliced load+round so the first projection matmuls start after one
    # 512-col slice instead of the whole 4 MB x transfer (~19us startup)
    x_ld = tmp.tile([P, 2, S], F32, tag="big")  # xf as two 128-row chunks
    x_sb = consts.tile([P, 2, S], F32R)
    xr = xf.rearrange("(c p) s -> p c s", p=P)
    for sl in range(NQ):
        nc.sync.dma_start(out=x_ld[:, :, bass.ts(sl, QC)],
                          in_=xr[:, :, bass.ts(sl, QC)])
        nc.vector.tensor_copy(x_sb[:, :, bass.ts(sl, QC)],
                              x_ld[:, :, bass.ts(sl, QC)])
    bq_sb = consts.tile([P, 1], F32)
    nc.sync.dma_start(out=bq_sb, in_=bq)
    bk_sb = consts.tile([P, 1], F32)
    nc.sync.dma_start(out=bk_sb, in_=bk)
    bv_row = consts.tile([1, P], F32)
    nc.sync.dma_start(out=bv_row, in_=bv)
    mask_ld = tmp.tile([P, 4, QC], F32, tag="w2")
    nc.sync.dma_start(out=mask_ld, in_=masks.rearrange("m p q -> p m q"))
    mask_sb = consts.tile([P, 4, QC], BF16)
    nc.vector.tensor_copy(mask_sb, mask_ld)

    bv_bc = consts.tile([P, P], F32)            # bv broadcast down partitions
    nc.gpsimd.partition_broadcast(bv_bc, bv_row)

    # On-device repeat loop for timing runs (reps>1): the whole compute
    # phase re-executes; consts/DMA loads stay outside.
    if reps > 1:
        loop_cm = tc.For_i(0, reps, 1)
        loop_cm.__enter__()

    # ---- q/k projections: qT/kT = W.T-slice @ xf + bias ----------------
    qT = consts.tile([P, S], BF16)               # 4 heads x 32 dims on partitions
    kT = consts.tile([P, S], BF16)
    for dst, w_sb, b_sb in ((qT, wq_sb, bq_sb), (kT, wk_sb, bk_sb)):
        for j in range(NQ):
            ps = qk_ps.tile([P, W_BANKS * QC], F32, tag="qk")
            for cc in range(2):
                nc.tensor.matmul(ps[:, 0:QC], w_sb[:, cc, :],
                                 x_sb[:, cc, bass.ts(j, QC)],
                                 start=(cc == 0), stop=(cc == 1))
            nc.scalar.activation(dst[:, bass.ts(j, QC)], ps[:, 0:QC],
                                 Ident, bias=b_sb, scale=1.0)

    # ---- v projection, position-major: v[s, o] for our 4 heads ---------
    # One k-tile of 128 positions per matmul; N=256 (all 8 heads) keeps
    # float32r at full rate; we keep only our head-group's 128 columns.
    # v_all[:, st, 34h:34h+33]: (128 kpos, 33) per (k-tile, head):
    # cols 0:32 = v, col 32 = 1.0 (softmax denominator via AV matmul).
    v_all = consts.tile([P, S // KT, HG * VW], BF16)
    for h in range(HG):
        nc.vector.memset(v_all[:, :, h * VW + 32:h * VW + 34], 1.0)
    # wvT columns are pre-rotated on the host so this core's head-group
    # occupies columns 0:128 of the v projection output.
    for st in range(S // KT):
        ps = qk_ps.tile([P, W_BANKS * QC], F32, tag="qk")
        for cc in range(2):
            nc.tensor.matmul(ps[:, 0:O], x_sb[:, cc, bass.ts(st, KT)],
                             wv_sb[:, cc, :], start=(cc == 0),
                             stop=(cc == 1))
        vs = v_all[:, st, :]
        nc.vector.tensor_add(_ap3(vs, VW, HG, 1, HD),
                             _ap3(ps[:, 0:P], HD, HG, 1, HD),
                             _ap3(bv_bc[:, 0:P], HD, HG, 1, HD))

    # Matmul operands cannot start at partition 96 (PE quadrant-3 weight
    # feed is unsupported), so head 3's q/k rows get their own partition-0
    # tiles.
    q3k3 = tmp.tile([HD, 2, S], BF16, tag="big2")
    nc.vector.tensor_copy(q3k3[:, 0, :], qT[3 * HD:4 * HD, :])
    nc.vector.tensor_copy(q3k3[:, 1, :], kT[3 * HD:4 * HD, :])

    # ---- attention ------------------------------------------------------
    # Emission in window-groups of G: G windows of QK+exp(+mask), then those
    # windows' AV matmuls.  Grouping keeps the PE on long runs of one
    # stationary shape (QK vs AV weight loads serialize when alternating),
    # and the one-group skew lets ScalarE exp run concurrently with both.
    G = 4
    outn = consts.tile([P, S], F32R)             # normalized out^T, 4h x 32dv
    # Pack k-tiles into psum windows by column width (<= 1536).  Diagonal
    # tiles (kt >= 4j) shrink to their causally-valid columns [t*128:512]
    # and are emitted FIRST (t0 covers the full 512 so its start=True matmul
    # resets the whole accumulator), packed [512][384|128][256] with no
    # holes.  Full tiles follow, 3 per window.
    # Matmul psum writes cannot cross a 512-col bank boundary.
    windows = []                                 # (h, j, is_last, [(kt, c0, off)])
    for h in range(HG):
        for j in range(NQ):
            diag = [(4 * j + t, t * KT) for t in (0, 1, 3, 2)]
            fulls = [(kt, 0) for kt in range(4 * j)]
            order = diag + fulls
            cur, off = [], 0
            for kt, c0 in order:
                w = QC - c0
                if (off // QC) != ((off + w - 1) // QC):
                    off = ((off + QC - 1) // QC) * QC
                if off + w > W_BANKS * QC:
                    windows.append((h, j, False, cur))
                    cur, off = [], 0
                cur.append((kt, c0, off))
                off += w
            windows.append((h, j, True, cur))
    groups = [windows[g0:g0 + G] for g0 in range(0, len(windows), G)]

    # bf16-Schraudolph constants for the DVE/Pool exp path
    sch_b = consts.tile([P, 1], F32)
    nc.vector.memset(sch_b, SCHRAUD_B)

    av_tiles = {}
    av_done = {}
    exp_rr = [0]

    def emit_qk_grp(grp):
        ess = []
        for (h, j, is_last, tiles) in grp:
            qh = qT[h * HD:(h + 1) * HD, :] if h < 3 else q3k3[:, 0, :]
            kh = kT[h * HD:(h + 1) * HD, :] if h < 3 else q3k3[:, 1, :]
            ps = qk_ps.tile([P, W_BANKS * QC], F32, tag="qk", name="ps")
            segs = []
            for (kt, c0, off) in tiles:
                segs.append((off, off + QC - c0))
                nc.tensor.matmul(ps[:, off:off + QC - c0],
                                 kh[:, bass.ts(kt, KT)],
                                 qh[:, j * QC + c0:(j + 1) * QC],
                                 start=True, stop=True)
            # merge written segments into contiguous spans (packing may leave
            # bank-alignment holes that must not be read)
            segs.sort()
            spans = [list(segs[0])]
            for s0, s1 in segs[1:]:
                if s0 == spans[-1][1]:
                    spans[-1][1] = s1
                else:
                    spans.append([s0, s1])
            es = work.tile([P, W_BANKS * QC], BF16, tag="es", name="es", bufs=5)
            eng = EXP_PATTERN[exp_rr[0] % len(EXP_PATTERN)]
            exp_rr[0] += 1
            for s0, s1 in spans:
                if eng == "A":
                    nc.scalar.activation(es[:, s0:s1], ps[:, s0:s1],
                                         Exp, scale=scale)
                else:
                    sb_bc = bass.AP(tensor=sch_b.tensor, offset=sch_b.offset,
                                    ap=[sch_b.ap[0], [0, s1 - s0]])
                    stt = (nc.vector.scalar_tensor_tensor if eng == "D"
                           else nc.gpsimd.scalar_tensor_tensor)
                    stt(es[:, s0:s1].bitcast(I16), ps[:, s0:s1],
                        SCHRAUD_A, sb_bc,
                        op0=mybir.AluOpType.mult, op1=mybir.AluOpType.add)
            # causal mask post-exp (multiply by 0/1): off the QK->exp path
            for (kt, c0, off) in tiles:
                if kt >= 4 * j:
                    sl = es[:, off:off + QC - c0]
                    nc.vector.tensor_mul(sl, sl,
                                         mask_sb[:, kt - 4 * j, c0:QC])
            ess.append(es)
        return ess

    def emit_av_grp(grp, ess):
        for es, (h, j, is_last, tiles) in zip(ess, grp):
            if (h, j) not in av_tiles:
                av_tiles[(h, j)] = av_ps.tile([33, QC], F32, tag="av",
                                              name="av")
                av_done[(h, j)] = 0
            av = av_tiles[(h, j)]
            ntot = 4 * j + 4
            for (kt, c0, off) in tiles:
                av_done[(h, j)] += 1
                nc.tensor.matmul(av[:, c0:QC],
                                 v_all[:, kt, h * VW:h * VW + 33],
                                 es[:, off:off + QC - c0],
                                 start=(av_done[(h, j)] == 1),
                                 stop=(av_done[(h, j)] == ntot))
            if is_last:
                # quick PSUM evacuation (frees the accumulator bank before
                # the serial normalization chain), then normalize rows 0:32
                # by the denominator row 32.  The recip custom-DVE op needs
                # an SBUF input on hardware.
                avs = norm.tile([32, QC], F32, tag="avs", name="avs")
                nc.vector.tensor_copy(avs, av[0:32, :])
                l0 = norm.tile([1, QC], F32, tag="l0", name="l0")
                nc.vector.tensor_copy(l0, av[32:33, :])
                recip = norm.tile([1, QC], F32, tag="recip", name="recip")
                nc.vector.reciprocal_approx_fast(recip, l0)
                rbc = norm.tile([32, QC], F32, tag="rbc", name="rbc")
                nc.gpsimd.partition_broadcast(rbc, recip)
                nc.gpsimd.tensor_mul(outn[h * HD:(h + 1) * HD, bass.ts(j, QC)],
                                     avs, rbc)
                del av_tiles[(h, j)]
                del av_done[(h, j)]

    for grp in groups:
        ess = emit_qk_grp(grp)
        emit_av_grp(grp, ess)

    # ---- output projection: out = Wp[:, our 128 cols] @ outn ----------
    for j in range(NQ):
        for m in range(2):
            ps = qk_ps.tile([P, W_BANKS * QC], F32, tag="qk")
            nc.tensor.matmul(ps[:, 0:QC], wp_sb[:, m, :],
                             outn[:, bass.ts(j, QC)],
                             start=True, stop=True)
            ob = work.tile([P, QC], F32, tag="ob", bufs=4)
            nc.scalar.activation(ob, ps[:, 0:QC], Ident, bias=0.0, scale=1.0)
            nc.sync.dma_start(
                out=out.rearrange("(m p) s -> p m s", p=P)[:, m,
                                                           bass.ts(j, QC)],
                in_=ob)

    if reps > 1:
        loop_cm.__exit__(None, None, None)


_BUILT = {}


def _get_built(reps=1):
    if reps not in _BUILT:
        _BUILT[reps] = build_kernel(reps)
    return _BUILT[reps]


def make_in_maps(x, Wq, bq, Wkv, bkv, Wp, bp):
    x = np.asarray(x, dtype=np.float32)
    Wq = np.asarray(Wq, dtype=np.float32)
    bq = np.asarray(bq, dtype=np.float32)
    Wkv = np.asarray(Wkv, dtype=np.float32)
    bkv = np.asarray(bkv, dtype=np.float32)
    Wp = np.asarray(Wp, dtype=np.float32)

    Wk, Wv = Wkv[:E], Wkv[E:]
    bk_, bv_ = bkv[:E], bkv[E:]

    # causal masks in transposed-score orientation (kpos partition, qpos free)
    kk = np.arange(P)[:, None]
    qq = np.arange(QC)[None, :]
    mask_np = np.stack([
        (qq >= d0 + kk).astype(np.float32)
        for d0 in (0, 128, 256, 384)])

    in_maps = []
    for c in range(N_CORES):
        n, hg = c // 2, c % 2
        rows = slice(hg * P, (hg + 1) * P)
        # rotate wvT columns so this core's 128 head columns sit at 0:128
        wvT_c = np.ascontiguousarray(np.roll(Wv.T, -hg * P, axis=1))
        in_maps.append({
            "xf": np.ascontiguousarray(x[n].reshape(C, S)),
            "wqT": np.ascontiguousarray(Wq[rows].T),
            "wkT": np.ascontiguousarray(Wk[rows].T),
            "wvT": wvT_c,
            "wpT": np.ascontiguousarray(
                Wp[:, rows].reshape(2, P, P).transpose(0, 2, 1)),
            "bq": np.ascontiguousarray(bq[rows, None]),
            "bk": np.ascontiguousarray(bk_[rows, None]),
            "bv": np.ascontiguousarray(bv_[None, rows]),
            "masks": mask_np,
        })
    return in_maps


def kernel(x, Wq, bq, Wkv, bkv, Wp, bp, n_heads):
    assert int(n_heads) == H
    bp = np.asarray(bp, dtype=np.float32)

    from concourse.bass_utils import run_bass_kernel_spmd

    nc = _get_built()
    in_maps = make_in_maps(x, Wq, bq, Wkv, bkv, Wp, bp)

    res = run_bass_kernel_spmd(nc, in_maps, core_ids=list(range(N_CORES)))

    outp = np.zeros((N, O, S), np.float32)
    for c in range(N_CORES):
        outp[c // 2] += res.results[c]["out"]
    outp += bp[None, :, None]
    return outp.reshape(N, O, HH, WW)
